# revision 13
# baseline (speedup 1.0000x reference)
"""MiTStage (involution patch-embed + 2 Mamba blocks) Trainium2 kernel.

Sharding: 8 cores = 4 batches x 2 d_inner-halves (128 channels each).
Per-core layout is feature-major: activations live as (feature partitions,
token columns). Each core computes the full xc (both halves) locally so
x_proj needs no collective; only the out_proj partial is pair-AllReduced.

v2: scan stage restructured — selective-scan y accumulation moved to the
PE (identity-matmul PSUM accumulate), B/C broadcasts consumed directly
from PSUM, z kept in SBUF (fp16), rsqrt via scalar Ln/Exp (no slow
1-partition reciprocal), conv + misc elementwise moved to the Pool
engine, fp16 weights/moving operands for the big matmuls.

Self-contained: hardcodes all shapes; host side only slices/transposes
weights and assembles the output.
"""

import os

import numpy as np

import concourse.bass as bass
import concourse.mybir as mybir
from concourse.tile import TileContext
from concourse.bass_utils import run_bass_kernel_spmd

AF = mybir.ActivationFunctionType
OP = mybir.AluOpType
FP32 = mybir.dt.float32
FP16 = mybir.dt.float16
F32R = mybir.dt.float32r

B, CIN, H, W = 4, 64, 128, 128
E, DEPTH = 128, 2
DD, NS, RR, HM = 256, 16, 8, 512  # d_inner, d_state, dt_rank, mlp hidden
HO = WO = 64
NT = HO * WO  # 4096 tokens
DH = DD // 2  # 128 channels per core
GROUPS = [[0, 1], [2, 3], [4, 5], [6, 7]]
MMN = 512     # matmul moving-dim chunk
TC = 1024     # stage-A token chunk
TS = 512      # scan-stage token chunk (PSUM-bank limited)
TCM = 512     # mlp token chunk

# dtype switches for the scan inner loop (flip after micro-benchmarks)
SCAN_B16 = os.environ.get("KB16", "0") == "1"   # b operand of scan in fp16
SCAN_O16 = os.environ.get("KO16", "0") == "1"   # scan output in fp16
CONV_VEC = os.environ.get("KCONVV", "1") == "1"  # conv on vector instead of pool


def _split_multiwaits(nc):
    """This container's walrus allows only one sem-wait per instruction;
    hoist extra waits onto same-engine NOPs inserted just before."""
    f = nc.m.functions[0]
    for blk in f.blocks:
        out = []
        changed = False
        for i in blk.instructions:
            si = i.sync_info
            if si and si.on_wait and len(si.on_wait) > 1:
                waits = list(si.on_wait)
                for k, wt in enumerate(waits[:-1]):
                    nop = mybir.InstNoOp(name=f"{i.name}_wsplit{k}")
                    nop.engine = i.engine
                    nop.sync_info = mybir.SyncInfo(on_wait=[wt], on_update=[])
                    out.append(nop)
                si.on_wait = [waits[-1]]
                changed = True
            out.append(i)
        if changed:
            blk.instructions = out


def _chunks(n_total, chunk):
    return [(c, min(chunk, n_total - c)) for c in range(0, n_total, chunk)]


def _build(debug=False):
    stage_lim = os.environ.get("KSTAGES", "full")
    nc = bass.Bass("TRN2", num_devices=8)

    def din(name, shape):
        return nc.dram_tensor(name, list(shape), FP32, kind="ExternalInput")

    x_pad = din("x_pad", (CIN, H + 2, W + 2))
    inv_rw = din("inv_rw", (CIN, 17))          # reduce_w.T / 4, col16=0
    inv_rb = din("inv_rb", (17, 1))           # row16 = 1.0
    span_rep = din("span_rep", (17, 9, CIN))   # [span_w[k]; span_b[k]] bcast M
    projw = din("projw", (CIN, E))             # proj_w.T
    bns = din("bns", (E, 1))
    bnb = din("bnb", (E, 1))
    dper = {}
    for i in range(DEPTH):
        dper[i] = {
            "n1w": din(f"n1w_{i}", (E, 1)),
            "ipx": din(f"ipx_{i}", (E, DD)),      # full in_proj xc rows, T
            "ipz": din(f"ipz_{i}", (E, DH)),      # z rows for this half, T
            "cw": din(f"cw_{i}", (DD, 4)),        # full conv (permuted)
            "cb": din(f"cb_{i}", (DD, 1)),
            "xpw": din(f"xpw_{i}", (DD, RR + 2 * NS)),  # full x_proj.T
            "dtw": din(f"dtw_{i}", (RR, DH)),
            "dtb": din(f"dtb_{i}", (DH, 1)),
            "A": din(f"A_{i}", (DH, NS)),
            "Dd": din(f"Dd_{i}", (DH, 1)),
            "opw": din(f"opw_{i}", (DH, E)),
            "n2w": din(f"n2w_{i}", (E, 1)),
            "f1w": din(f"f1w_{i}", (E, HM)),
            "f1b": din(f"f1b_{i}", (E, HM // E)),
            "f2w": din(f"f2w_{i}", (HM, E)),
            "f2b": din(f"f2b_{i}", (E, 1)),
        }
    y_out = nc.dram_tensor("y_out", [E, NT], FP32, kind="ExternalOutput")

    dbg = {}
    if debug:
        for nm, shape in [("t0", (E, NT)), ("xc0", (DH, NT)), ("dbl0", (40, NT)),
                          ("dt0", (DH, NT)), ("yacc0", (DH, NT)), ("t1", (E, NT))]:
            dbg[nm] = nc.dram_tensor(f"dbg_{nm}", list(shape), FP32,
                                     kind="ExternalOutput")

    ones_row_d = nc.inline_tensor(np.ones((1, E), np.float32), name="ones_row")
    ones_col_d = nc.inline_tensor(np.ones((E, 1), np.float32), name="ones_colv")
    ident16_d = nc.inline_tensor(np.eye(128, dtype=np.float16), name="ident16")
    all16_d = nc.inline_tensor(np.ones((128, 128), np.float16), name="all16")

    ccy = [(nc.dram_tensor(f"ccyi_{i}", [E, NT], FP32),
            nc.dram_tensor(f"ccyo_{i}", [E, NT], FP32)) for i in range(DEPTH)]

    with TileContext(nc) as tc:
        with tc.tile_pool(name="persist", bufs=1) as pp:
            t_res = pp.tile([E, NT], FP32, tag="t_res")
            ones_r = pp.tile([1, E], F32R, tag="ones_r")
            ones_c = pp.tile([E, 1], F32R, tag="ones_c")
            idt16 = pp.tile([128, 128], FP16, tag="idt16")
            all16 = pp.tile([128, 128], FP16, tag="all16")
            eps_t = pp.tile([1, 1], FP32, tag="eps_t")
            nc.vector.memset(eps_t[:], 1e-5)
            orf = pp.tile([1, E], FP32, tag="orf")
            ocf = pp.tile([E, 1], FP32, tag="ocf")
            nc.sync.dma_start(orf[:], ones_row_d[:])
            nc.sync.dma_start(ocf[:], ones_col_d[:])
            nc.sync.dma_start(idt16[:], ident16_d[:])
            nc.sync.dma_start(all16[:], all16_d[:])
            nc.vector.tensor_copy(ones_r[:], orf[:])
            nc.vector.tensor_copy(ones_c[:], ocf[:])

            # =================== Stage A: involution ===================
            with tc.tile_pool(name="sA", bufs=2) as sa, \
                 tc.tile_pool(name="sA1", bufs=1) as sa1, \
                 tc.tile_pool(name="pA", bufs=2, space="PSUM") as pa:
                xt = sa1.tile([CIN, H + 2, W + 2], FP32, tag="xt")
                nc.sync.dma_start(xt[:], x_pad[:])
                rwf = sa1.tile([CIN, 17], FP32, tag="rwf")
                rb = sa1.tile([17, 1], FP32, tag="rb")
                spwf = sa1.tile([17, 9, CIN], FP32, tag="spwf")
                pwf = sa1.tile([CIN, E], FP32, tag="pwf")
                bs = sa1.tile([E, 1], FP32, tag="bns")
                bbt = sa1.tile([E, 1], FP32, tag="bnb")
                for src, dst in [(inv_rw, rwf), (inv_rb, rb), (span_rep, spwf),
                                 (projw, pwf), (bns, bs), (bnb, bbt)]:
                    nc.sync.dma_start(dst[:], src[:])
                rw = sa1.tile([CIN, 17], F32R, tag="rw")
                spw = sa1.tile([17, 9, CIN], F32R, tag="spw")
                pw = sa1.tile([CIN, E], F32R, tag="pw")
                nc.vector.tensor_copy(rw[:], rwf[:])
                nc.vector.tensor_copy(spw[:], spwf[:])
                nc.vector.tensor_copy(pw[:], pwf[:])

                xin = xt[:, 1:H + 1, 1:W + 1]
                p1 = sa1.tile([CIN, H, WO], FP32, tag="p1")
                nc.vector.tensor_tensor(p1[:], xin[:, :, 0:W:2], xin[:, :, 1:W:2],
                                        OP.add)
                xk = sa1.tile([CIN, HO, WO], F32R, tag="xk")
                nc.vector.tensor_tensor(xk[:], p1[:, 0:H:2, :], p1[:, 1:H:2, :],
                                        OP.add)
                # hid = relu(rw.T @ xk + rb); rw col16=0, rb row16=1.0 so
                # hid row 16 == relu(0+1) == 1.0 (bias row for span matmul)
                hid = sa1.tile([17, NT], F32R, tag="hid")
                xkf = xk.rearrange("c a b -> c (a b)")
                for c0, cn in _chunks(NT, MMN):
                    ps = pa.tile([17, MMN], FP32, tag="ps_hid")
                    nc.tensor.matmul(ps[:, :cn], rw[:], xkf[:, c0:c0 + cn],
                                     start=True, stop=True)
                    nc.scalar.activation(hid[:, c0:c0 + cn], ps[:, :cn], AF.Relu,
                                         bias=rb[:])
                for c0, cn in _chunks(NT, TC):
                    ho0 = c0 // WO
                    hon = cn // WO
                    vch = sa.tile([CIN, TC], F32R, tag="vch")
                    for k in range(9):
                        di, dj = k // 3, k % 3
                        kb = pa.tile([CIN, TC], FP32, tag="kb")
                        for m0, mn in _chunks(cn, MMN):
                            nc.tensor.matmul(kb[:, m0:m0 + mn], spw[:, k, :],
                                             hid[:, c0 + m0:c0 + m0 + mn],
                                             start=True, stop=True)
                        xs = xt[:, di + 2 * ho0: di + 2 * (ho0 + hon): 2,
                                dj: dj + W: 2]
                        kb3 = kb[:, :cn].rearrange("c (a b) -> c a b", b=WO)
                        vch3 = vch[:, :cn].rearrange("c (a b) -> c a b", b=WO)
                        if k == 0:
                            nc.vector.tensor_tensor(vch3, kb3, xs, OP.mult)
                        else:
                            tmp = sa.tile([CIN, TC], FP32, tag="kbtmp")
                            tmp3 = tmp[:, :cn].rearrange("c (a b) -> c a b", b=WO)
                            nc.vector.tensor_tensor(tmp3, kb3, xs, OP.mult)
                            nc.vector.tensor_tensor(vch[:, :cn], vch[:, :cn],
                                                    tmp[:, :cn], OP.add)
                    for m0, mn in _chunks(cn, MMN):
                        ps = pa.tile([E, MMN], FP32, tag="ps_proj")
                        nc.tensor.matmul(ps[:, :mn], pw[:], vch[:, m0:m0 + mn],
                                         start=True, stop=True)
                        nc.scalar.activation(t_res[:, c0 + m0:c0 + m0 + mn],
                                             ps[:, :mn], AF.Identity,
                                             bias=bbt[:], scale=bs[:])
            if debug:
                nc.sync.dma_start(dbg["t0"][:], t_res[:])

            # =================== Stage B: depth blocks ===================
            depth_range = [] if stage_lim == "A" else (
                [0] if stage_lim.startswith("D0") else list(range(DEPTH)))
            for i in depth_range:
                wd = dper[i]
                with tc.tile_pool(name=f"w{i}", bufs=1) as wp:
                    w = {}
                    for nm in ["n1w", "dtb", "A", "Dd", "n2w", "f1b", "f2b"]:
                        w[nm] = wp.tile(list(wd[nm].shape), FP32, tag=nm,
                                        name=f"{nm}_{i}")
                        nc.sync.dma_start(w[nm][:], wd[nm][:])
                    cw_h = []
                    cb_h = []
                    for hh in range(2):
                        cwt = wp.tile([DH, 4], FP32, tag=f"cw{hh}",
                                      name=f"cw{hh}_{i}")
                        nc.sync.dma_start(cwt[:], wd["cw"][hh * DH:(hh + 1) * DH, :])
                        cw_h.append(cwt)
                        cbt = wp.tile([DH, 1], FP32, tag=f"cb{hh}",
                                      name=f"cb{hh}_{i}")
                        nc.sync.dma_start(cbt[:], wd["cb"][hh * DH:(hh + 1) * DH, :])
                        cb_h.append(cbt)

                    def load_w(nm, shape, dt, slc=None, tagsuf="", i=i):
                        tf = wp.tile(list(shape), FP32, tag=f"{nm}{tagsuf}f",
                                     name=f"{nm}{tagsuf}f_{i}")
                        nc.sync.dma_start(tf[:], wd[nm][:] if slc is None
                                          else wd[nm][slc])
                        tr = wp.tile(list(shape), dt, tag=f"{nm}{tagsuf}r",
                                     name=f"{nm}{tagsuf}r_{i}")
                        nc.vector.tensor_copy(tr[:], tf[:])
                        return tr

                    ipx0 = load_w("ipx", (E, DH), FP16, np.s_[:, 0:DH], "0")
                    ipx1 = load_w("ipx", (E, DH), FP16, np.s_[:, DH:DD], "1")
                    ipz = load_w("ipz", (E, DH), FP16)
                    xpw0 = load_w("xpw", (DH, RR + 2 * NS), FP16, np.s_[0:DH, :],
                                  "0")
                    xpw1 = load_w("xpw", (DH, RR + 2 * NS), FP16, np.s_[DH:DD, :],
                                  "1")
                    dtw = load_w("dtw", (RR, DH), FP16)
                    opw = load_w("opw", (DH, E), F32R)
                    f1w = load_w("f1w", (E, HM), FP16)
                    f2t = [load_w("f2w", (E, E), FP16,
                                  np.s_[kt * E:(kt + 1) * E, :], str(kt))
                           for kt in range(HM // E)]

                    with tc.tile_pool(name=f"mx{i}", bufs=1) as mp, \
                         tc.tile_pool(name=f"hn{i}", bufs=1) as hnp:
                        # ---- rms1 + normed h (fp16) ----
                        hn = hnp.tile([E, NT], FP16, tag="hn")
                        with tc.tile_pool(name=f"r{i}", bufs=2) as rp, \
                             tc.tile_pool(name=f"r1{i}", bufs=1) as rp1, \
                             tc.tile_pool(name=f"pr{i}", bufs=2, space="PSUM") as pr:
                            rs = rp1.tile([1, NT], F32R, tag="rs")
                            for c0, cn in _chunks(NT, MMN):
                                sq = rp.tile([E, MMN], F32R, tag="sq")
                                nc.scalar.activation(sq[:, :cn],
                                                     t_res[:, c0:c0 + cn], AF.Square)
                                ps = pr.tile([1, MMN], FP32, tag="ps_rs")
                                nc.tensor.matmul(ps[:, :cn], ones_c[:],
                                                 sq[:, :cn], start=True, stop=True)
                                # ln(mean_sq + eps)
                                nc.scalar.activation(rs[:, c0:c0 + cn], ps[:, :cn],
                                                     AF.Ln, scale=1.0 / E,
                                                     bias=eps_t[:])
                            # rsqrt = exp(-0.5 * ln(...))
                            nc.scalar.activation(rs[:], rs[:], AF.Exp, scale=-0.5)
                            for c0, cn in _chunks(NT, MMN):
                                inv = pr.tile([E, MMN], FP32, tag="ps_inv")
                                nc.tensor.matmul(inv[:, :cn], ones_r[:],
                                                 rs[:, c0:c0 + cn], start=True,
                                                 stop=True)
                                nc.vector.scalar_tensor_tensor(
                                    hn[:, c0:c0 + cn], t_res[:, c0:c0 + cn],
                                    w["n1w"][:], inv[:, :cn], OP.mult, OP.mult)
                        # ---- in_proj (full xc + z half) + conv + silu ----
                        xc16 = [mp.tile([DH, NT], FP16, tag="xc16_0",
                                        name=f"xc16_0_{i}"),
                                mp.tile([DH, NT], FP16, tag="xc16_1",
                                        name=f"xc16_1_{i}")]
                        z16 = mp.tile([DH, NT], FP16, tag="z16", name=f"z16_{i}")
                        with tc.tile_pool(name=f"ip{i}", bufs=1) as cp, \
                             tc.tile_pool(name=f"pip{i}", bufs=2, space="PSUM") as pip:
                            xcp = [cp.tile([DH, NT + 3], FP16, tag="xcp0",
                                           name=f"xcp0_{i}"),
                                   cp.tile([DH, NT + 3], FP16, tag="xcp1",
                                           name=f"xcp1_{i}")]
                            nc.vector.memset(xcp[0][:, 0:3], 0)
                            nc.vector.memset(xcp[1][:, 0:3], 0)
                            for c0, cn in _chunks(NT, MMN):
                                for hh, ipx in ((0, ipx0), (1, ipx1)):
                                    ps = pip.tile([DH, MMN], FP32,
                                                  tag=f"ps_ip{hh}",
                                                  name=f"ps_ip{hh}_{i}_{c0}")
                                    nc.tensor.matmul(ps[:, :cn], ipx,
                                                     hn[:, c0:c0 + cn],
                                                     start=True, stop=True)
                                    nc.scalar.copy(xcp[hh][:, 3 + c0:3 + c0 + cn],
                                                   ps[:, :cn])
                                ps2 = pip.tile([DH, MMN], FP32, tag="ps_ipz")
                                nc.tensor.matmul(ps2[:, :cn], ipz,
                                                 hn[:, c0:c0 + cn], start=True,
                                                 stop=True)
                                nc.scalar.copy(z16[:, c0:c0 + cn], ps2[:, :cn])
                            # conv (fp16) on pool (or vector fallback)
                            ceng = nc.vector if CONV_VEC else nc.gpsimd
                            for hh in range(2):
                                cv = cp.tile([DH, NT], FP16, tag=f"cv{hh}",
                                             name=f"cv{hh}_{i}")
                                ceng.tensor_scalar(
                                    out=cv[:], in0=xcp[hh][:, 0:NT],
                                    scalar1=cw_h[hh][:, 0:1], scalar2=None,
                                    op0=OP.mult)
                                for j in range(1, 4):
                                    ceng.scalar_tensor_tensor(
                                        cv[:], xcp[hh][:, j:j + NT],
                                        cw_h[hh][:, j:j + 1], cv[:],
                                        OP.mult, OP.add)
                                nc.scalar.activation(xc16[hh][:], cv[:], AF.Silu,
                                                     bias=cb_h[hh][:])
                        # ---- x_proj (full, local) + dt ----
                        sp2cm = tc.tile_pool(name=f"sp2{i}", bufs=1)
                        sp2 = sp2cm.__enter__()
                        dbl16 = sp2.tile([40, NT], FP16, tag="dbl16")
                        dt32 = sp2.tile([DH, NT], FP32, tag="dt32")
                        duc16 = sp2.tile([DH, NT], FP16, tag="duc16")
                        with tc.tile_pool(name=f"pxp{i}", bufs=2, space="PSUM") as pxp:
                            for c0, cn in _chunks(NT, MMN):
                                ps = pxp.tile([40, MMN], FP32, tag="ps_xp")
                                nc.tensor.matmul(ps[:, :cn], xpw0,
                                                 xc16[0][:, c0:c0 + cn],
                                                 start=True, stop=False)
                                nc.tensor.matmul(ps[:, :cn], xpw1,
                                                 xc16[1][:, c0:c0 + cn],
                                                 start=False, stop=True)
                                nc.scalar.copy(dbl16[:, c0:c0 + cn], ps[:, :cn])
                        with tc.tile_pool(name=f"pdt{i}", bufs=2, space="PSUM") as pdt:
                            for c0, cn in _chunks(NT, MMN):
                                ps = pdt.tile([DH, MMN], FP32, tag="ps_dt")
                                nc.tensor.matmul(ps[:, :cn], dtw,
                                                 dbl16[0:RR, c0:c0 + cn],
                                                 start=True, stop=True)
                                nc.scalar.activation(dt32[:, c0:c0 + cn],
                                                     ps[:, :cn],
                                                     AF.Exp, bias=w["dtb"][:])
                        nc.scalar.activation(dt32[:], dt32[:], AF.Ln, bias=1.0)
                        # duc = dt * xc (local half)
                        nc.vector.tensor_tensor(duc16[:], dt32[:],
                                                xc16[0][:], OP.mult)
                        if debug and i == 0:
                            dblf = sp2.tile([40, NT], FP32, tag="dblf_dbg")
                            nc.vector.tensor_copy(dblf[:], dbl16[:])
                            nc.sync.dma_start(dbg["dbl0"][:], dblf[:])
                            nc.sync.dma_start(dbg["dt0"][:], dt32[:])

                        # ---- selective scan (PE-accumulated y) ----
                        hstate = sp2.tile([DH, NS], FP32, tag="hstate")
                        yin, yout = ccy[i]
                        yac_dbg = None
                        if debug and i == 0:
                            yac_dbg = sp2.tile([DH, NT], FP32, tag="yac_dbg")
                        # zeroed moving tiles for the K=128 all-ones broadcast:
                        # row 0 carries the B/C row, rows 1..127 stay zero.
                        mzt = [sp2.tile([128, 2, TS], FP16, tag=f"mzt{k}",
                                        name=f"mzt{k}_{i}") for k in range(2)]
                        nc.vector.memset(mzt[0][:], 0)
                        nc.vector.memset(mzt[1][:], 0)
                        with tc.tile_pool(name=f"sc{i}", bufs=2) as sp, \
                             tc.tile_pool(name=f"psc{i}", bufs=2, space="PSUM") as pscp, \
                             tc.tile_pool(name=f"psy{i}", bufs=2, space="PSUM") as pyp, \
                             tc.tile_pool(name=f"pg{i}", bufs=2, space="PSUM") as pgp, \
                             tc.tile_pool(name=f"g{i}", bufs=2) as gp:
                            n_ch = _chunks(NT, TS)
                            for ci, (c0, cn) in enumerate(n_ch):
                                psum_y = pyp.tile([DH, TS], FP32, tag="psum_y")
                                for n in range(NS):
                                    a_t = sp.tile([DH, TS], FP32, tag="a_t")
                                    nc.scalar.activation(a_t[:, :cn],
                                                         dt32[:, c0:c0 + cn],
                                                         AF.Exp,
                                                         scale=w["A"][:, n:n + 1])
                                    # stage B/C rows into row 0 of the zeroed
                                    # moving tile; K=128 all-ones matmul
                                    # broadcasts them to all partitions.
                                    mz = mzt[n % 2]
                                    nc.sync.dma_start(
                                        mz[0:1, 0, :cn],
                                        dbl16[RR + n:RR + n + 1, c0:c0 + cn])
                                    nc.sync.dma_start(
                                        mz[0:1, 1, :cn],
                                        dbl16[RR + NS + n:RR + NS + n + 1,
                                              c0:c0 + cn])
                                    bc_p = pscp.tile([DH, 2, TS], FP32,
                                                     tag="bc_p")
                                    nc.tensor.matmul(bc_p[:, 0, :cn], all16[:],
                                                     mz[:, 0, :cn],
                                                     start=True, stop=True)
                                    nc.tensor.matmul(bc_p[:, 1, :cn], all16[:],
                                                     mz[:, 1, :cn],
                                                     start=True, stop=True)
                                    b_t = sp.tile([DH, TS],
                                                  FP16 if SCAN_B16 else FP32,
                                                  tag="b_t")
                                    nc.vector.tensor_tensor(
                                        b_t[:, :cn], duc16[:, c0:c0 + cn],
                                        bc_p[:, 0, :cn], OP.mult)
                                    h_t = sp.tile([DH, TS],
                                                  FP16 if SCAN_O16 else FP32,
                                                  tag="h_t")
                                    init = 0.0 if ci == 0 else hstate[:, n:n + 1]
                                    nc.vector.tensor_tensor_scan(
                                        h_t[:, :cn], a_t[:, :cn], b_t[:, :cn],
                                        init, OP.mult, OP.add)
                                    if ci < len(n_ch) - 1:
                                        nc.vector.tensor_copy(
                                            hstate[:, n:n + 1], h_t[:, cn - 1:cn])
                                    hc = sp.tile([DH, TS], FP16, tag="hc")
                                    nc.vector.tensor_tensor(
                                        hc[:, :cn], h_t[:, :cn],
                                        bc_p[:, 1, :cn], OP.mult)
                                    nc.tensor.matmul(
                                        psum_y[:, :cn], idt16[:], hc[:, :cn],
                                        start=(n == 0), stop=(n == NS - 1))
                                # ---- gate + out_proj for this chunk ----
                                if yac_dbg is not None:
                                    nc.scalar.copy(yac_dbg[:, c0:c0 + cn],
                                                   psum_y[:, :cn])
                                sz = gp.tile([DH, TS], FP16, tag="sz")
                                nc.scalar.activation(sz[:, :cn],
                                                     z16[:, c0:c0 + cn], AF.Silu)
                                yd = gp.tile([DH, TS], F32R, tag="yd")
                                nc.vector.scalar_tensor_tensor(
                                    yd[:, :cn], xc16[0][:, c0:c0 + cn],
                                    w["Dd"][:], psum_y[:, :cn],
                                    OP.mult, OP.add)
                                yg = gp.tile([DH, TS], F32R, tag="yg")
                                nc.vector.tensor_tensor(yg[:, :cn], yd[:, :cn],
                                                        sz[:, :cn], OP.mult)
                                ps = pgp.tile([E, TS], FP32, tag="ps_op")
                                nc.tensor.matmul(ps[:, :cn], opw, yg[:, :cn],
                                                 start=True, stop=True)
                                st = gp.tile([E, TS], FP32, tag="st_op")
                                nc.scalar.copy(st[:, :cn], ps[:, :cn])
                                nc.sync.dma_start(yin[:, c0:c0 + cn],
                                                  st[:, :cn])
                        if debug and i == 0:
                            xcf = sp2.tile([DH, NT], FP32, tag="xcf_dbg")
                            nc.vector.tensor_copy(xcf[:], xc16[0][:])
                            nc.sync.dma_start(dbg["xc0"][:], xcf[:])
                            nc.sync.dma_start(dbg["yacc0"][:], yac_dbg[:])
                        sp2cm.__exit__(None, None, None)
                        nc.gpsimd.collective_compute(
                            "AllReduce", OP.add, GROUPS,
                            ins=[yin[:]], outs=[yout[:]])
                        with tc.tile_pool(name=f"ga{i}", bufs=2) as gap:
                            for c0, cn in _chunks(NT, TC):
                                opr = gap.tile([E, TC], FP32, tag="opr")
                                nc.sync.dma_start(opr[:, :cn], yout[:, c0:c0 + cn])
                                nc.vector.tensor_tensor(t_res[:, c0:c0 + cn],
                                                        t_res[:, c0:c0 + cn],
                                                        opr[:, :cn], OP.add)

                    # ---- MLP (redundant on both cores of the pair) ----
                    with tc.tile_pool(name=f"ml{i}", bufs=1) as lp:
                        rs2 = lp.tile([1, NT], F32R, tag="rs2")
                        with tc.tile_pool(name=f"mr{i}", bufs=2) as mrp, \
                             tc.tile_pool(name=f"pmr{i}", bufs=2, space="PSUM") as pmr:
                            for c0, cn in _chunks(NT, MMN):
                                sq = mrp.tile([E, MMN], F32R, tag="sq2")
                                nc.scalar.activation(sq[:, :cn],
                                                     t_res[:, c0:c0 + cn], AF.Square)
                                ps = pmr.tile([1, MMN], FP32, tag="ps_rs2")
                                nc.tensor.matmul(ps[:, :cn], ones_c[:], sq[:, :cn],
                                                 start=True, stop=True)
                                nc.scalar.activation(rs2[:, c0:c0 + cn], ps[:, :cn],
                                                     AF.Ln, scale=1.0 / E,
                                                     bias=eps_t[:])
                            nc.scalar.activation(rs2[:], rs2[:], AF.Exp, scale=-0.5)
                        with tc.tile_pool(name=f"mf{i}", bufs=2) as mfp, \
                             tc.tile_pool(name=f"pmf{i}", bufs=2, space="PSUM") as pmf:
                            for c0, cn in _chunks(NT, TCM):
                                inv = pmf.tile([E, TCM], FP32, tag="ps_inv2")
                                nc.tensor.matmul(inv[:, :cn], ones_r[:],
                                                 rs2[:, c0:c0 + cn], start=True,
                                                 stop=True)
                                h2 = mfp.tile([E, TCM], FP16, tag="h2")
                                nc.vector.scalar_tensor_tensor(
                                    h2[:, :cn], t_res[:, c0:c0 + cn], w["n2w"][:],
                                    inv[:, :cn], OP.mult, OP.mult)
                                gts = []
                                for mt in range(HM // E):
                                    ps = pmf.tile([E, TCM], FP32,
                                                  tag=f"ps_f1_{mt % 2}",
                                                  name=f"ps_f1_{mt}_{i}_{c0}")
                                    nc.tensor.matmul(
                                        ps[:, :cn], f1w[:, mt * E:(mt + 1) * E],
                                        h2[:, :cn], start=True, stop=True)
                                    gt = mfp.tile([E, TCM], FP16, tag=f"gt{mt}",
                                                  name=f"gt{mt}_{i}_{c0}")
                                    nc.scalar.activation(gt[:, :cn], ps[:, :cn],
                                                         AF.Gelu,
                                                         bias=w["f1b"][:, mt:mt + 1])
                                    gts.append(gt)
                                ps2 = pmf.tile([E, TCM], FP32, tag="ps_f2")
                                for kt in range(HM // E):
                                    nc.tensor.matmul(ps2[:, :cn], f2t[kt],
                                                     gts[kt][:, :cn],
                                                     start=(kt == 0),
                                                     stop=(kt == HM // E - 1))
                                mo = mfp.tile([E, TCM], FP32, tag="mo")
                                nc.scalar.activation(mo[:, :cn], ps2[:, :cn],
                                                     AF.Identity, bias=w["f2b"][:])
                                nc.vector.tensor_tensor(t_res[:, c0:c0 + cn],
                                                        t_res[:, c0:c0 + cn],
                                                        mo[:, :cn], OP.add)
                if debug and i == 0:
                    nc.sync.dma_start(dbg["t1"][:], t_res[:])

            nc.sync.dma_start(y_out[:], t_res[:])

    _split_multiwaits(nc)
    return nc


_CACHE = {}


def _get_nc(debug=False):
    key = (bool(debug), os.environ.get("KSTAGES", "full"),
           os.environ.get("KB16", "0"), os.environ.get("KO16", "0"))
    if key not in _CACHE:
        _CACHE[key] = _build(debug)
    return _CACHE[key]


def _host_inputs(inputs):
    """Build the 8 per-core input maps from full inputs.

    The device always scans xc16[0]; the host permutes the d_inner channel
    order so this core's half comes FIRST in ipx/cw/cb/xpw. A/dtw/dtb/Dd/
    opw use the unpermuted local half slice.
    """
    f = np.float32
    x = np.asarray(inputs["x"], f)
    x_pad = np.pad(x, ((0, 0), (0, 0), (1, 1), (1, 1)))
    reduce_w = np.asarray(inputs["reduce_w"], f)
    span_w = np.asarray(inputs["span_w"], f)
    span_b = np.asarray(inputs["span_b"], f)
    proj_w = np.asarray(inputs["proj_w"], f)
    bn_scale = (np.asarray(inputs["bn_gamma"], f)
                / np.sqrt(np.asarray(inputs["bn_var"], f) + 1e-5))
    bn_bias = (np.asarray(inputs["bn_beta"], f)
               - np.asarray(inputs["bn_mean"], f) * bn_scale)
    span_rep = np.empty((17, 9, CIN), f)
    for k in range(9):
        span_rep[:16, k] = span_w[k][:, None]
        span_rep[16, k] = span_b[k]

    inv_rw = np.zeros((CIN, 17), f)
    inv_rw[:, :16] = reduce_w.T / 4.0
    inv_rb = np.zeros((17, 1), f)
    inv_rb[:16, 0] = np.asarray(inputs["reduce_b"], f)
    inv_rb[16, 0] = 1.0
    common = {
        "inv_rw": inv_rw,
        "inv_rb": inv_rb,
        "span_rep": span_rep,
        "projw": proj_w.T.astype(f).copy(),
        "bns": bn_scale[:, None].astype(f),
        "bnb": bn_bias[:, None].astype(f),
    }
    in_proj_w = np.asarray(inputs["in_proj_w"], f)
    conv_w = np.asarray(inputs["conv_w"], f)
    conv_b = np.asarray(inputs["conv_b"], f)
    x_proj_w = np.asarray(inputs["x_proj_w"], f)
    dt_proj_w = np.asarray(inputs["dt_proj_w"], f)
    dt_proj_b = np.asarray(inputs["dt_proj_b"], f)
    A_full = -np.exp(np.asarray(inputs["A_log"], f))
    D_full = np.asarray(inputs["D"], f)
    out_proj_w = np.asarray(inputs["out_proj_w"], f)
    n1 = np.asarray(inputs["norm1_w"], f)
    n2 = np.asarray(inputs["norm2_w"], f)
    fc1_w = np.asarray(inputs["fc1_w"], f)
    fc1_b = np.asarray(inputs["fc1_b"], f)
    fc2_w = np.asarray(inputs["fc2_w"], f)
    fc2_b = np.asarray(inputs["fc2_b"], f)

    in_maps = []
    for core in range(8):
        b, r = core // 2, core % 2
        perm = np.r_[r * DH:(r + 1) * DH, (1 - r) * DH:(2 - r) * DH]
        sl = slice(r * DH, (r + 1) * DH)
        m = dict(common)
        m["x_pad"] = x_pad[b]
        for i in range(DEPTH):
            m[f"n1w_{i}"] = n1[i][:, None]
            m[f"ipx_{i}"] = in_proj_w[i][perm].T            # (E, DD) permuted
            m[f"ipz_{i}"] = in_proj_w[i][DD + r * DH:DD + (r + 1) * DH].T
            m[f"cw_{i}"] = conv_w[i][perm]
            m[f"cb_{i}"] = conv_b[i][perm][:, None]
            m[f"xpw_{i}"] = x_proj_w[i][:, perm].T          # (DD, 40) permuted
            m[f"dtw_{i}"] = dt_proj_w[i][sl].T
            m[f"dtb_{i}"] = dt_proj_b[i][sl][:, None]
            m[f"A_{i}"] = A_full[i][sl]
            m[f"Dd_{i}"] = D_full[i][sl][:, None]
            m[f"opw_{i}"] = out_proj_w[i][:, sl].T
            m[f"n2w_{i}"] = n2[i][:, None]
            m[f"f1w_{i}"] = fc1_w[i].T
            m[f"f1b_{i}"] = fc1_b[i].reshape(HM // E, E).T
            m[f"f2w_{i}"] = fc2_w[i].T
            m[f"f2b_{i}"] = fc2_b[i][:, None]
        m = {k: np.ascontiguousarray(v, f) for k, v in m.items()}
        in_maps.append(m)
    return in_maps


def kernel(_debug=False, _trace=False, _tmpdir=None, _trace_cores=None, **inputs):
    nc = _get_nc(_debug)
    in_maps = _host_inputs(inputs)
    kw = {}
    if _trace:
        kw.update(trace=True, tmpdir=_tmpdir, trace_cores=_trace_cores)
    res = run_bass_kernel_spmd(nc, in_maps, core_ids=list(range(8)), **kw)
    out = np.empty((B, E, HO, WO), np.float32)
    for b in range(B):
        out[b] = res.results[2 * b]["y_out"].reshape(E, HO, WO)
    if _debug or _trace:
        return out, res
    return out
